# revision 45
# baseline (speedup 1.0000x reference)
"""Trainium2 Bass kernel for Mesh2GridDecoder (GraphCast-style mesh->grid
message passing + output MLP), distributed over 8 NeuronCores.

Sharding: grid nodes (and edges, by destination) sharded across 8 cores so
the scatter-sum is core-local; mesh node features and weights replicated.

Math restructuring (exact, up to float re-association):
  h     = silu(attrs @ emb_w0 + emb_b0)                       per edge
  pre2  = mesh_proj[src] + grid_proj[dst] + h @ W_he
    with mesh_proj = mesh@Ws, grid_proj = grid@Wd + (emb_b1@We + edge_b0),
         W_he = emb_w1 @ We      (Ws|Wd|We = row blocks of edge_w0)
  hid2  = silu(pre2)
  pre3  = grid@W0a + (S@h)@U1 + (S@hid2)@U2 + cnt (x) v3 + node_b0
    with U1 = emb_w1@W0b, U2 = edge_w1@W0b, v3 = (emb_b1+edge_b1)@W0b
         (S = scatter-sum matrix, cnt = per-node edge count)
  hid3  = silu(pre3)
  pre4  = grid@out_w0 + hid3@V + b4,   V = node_w1@out_w0,
          b4 = node_b1@out_w0 + out_b0
  out   = silu(pre4) @ out_w1 + out_b1

Device structure per core:
  P1: mesh_proj = meshT @ Ws   (host pre-transposed mesh, plain DMA)
  P2: grid_proj = gridT @ Wd + b2 -> DRAM
  Fused edge+node phase per 512-row superblock (16 of them):
    per 128-dst block (4 per superblock), per 128-edge chunk:
      z  = attrs-chunk @ emb_w0  (PE, edge-major)
      hR = silu(z)               (scalar, fp8 out for the scatter)
      zT = XBAR DMA-transpose(z); hF = silu(zT)   (feature-major)
      pre2 = selT-mm(grid_proj block) + hF@W_he [DoubleRow fp8] in PSUM,
             + mesh_proj[src] (gpsimd gather) via one vector add
      hid2 = silu(pre2)          (scalar, fp8)
      scatter FEATURE-major: aggT[f,d] += hR/hid2 (lhsT) @ sel-chunk,
             fp8 DoubleRow over chunk pairs
    node+out MLPs on the 512 rows: weights feature-major, activations
    feature-major throughout; u1/u2/v matmuls in fp8 DoubleRow; biases
    folded into activations / the PSUM->SBUF copy.
"""
import math
import numpy as np
import ml_dtypes

import concourse.bass as bass
import concourse.tile as tile
from concourse import mybir
from concourse import bass_utils
from concourse import library_config
from concourse.vector_clock import ScopedClock

BF16 = mybir.dt.bfloat16
F32 = mybir.dt.float32
I16 = mybir.dt.int16
FP8 = mybir.dt.float8e4
AF = mybir.ActivationFunctionType
ALU = mybir.AluOpType
DR = mybir.MatmulPerfMode.DoubleRow
bf = ml_dtypes.bfloat16
f8 = ml_dtypes.float8_e4m3

N_MESH = 10242
N_GRID = 65160
N_EDGES = 195480
D = 512
OUTD = 471
NCORES = 8
GSH = N_GRID // NCORES          # 8145 grid rows per core
NGS = 8192                      # padded grid shard rows
NB = NGS // 128                 # 64 dst blocks per core
NSB = NB // 4                   # 16 superblocks (512 rows each)
NM = 11264                      # padded mesh rows (88 chunks of 128, /8 cores)
NMC = NM // 128
NMC_L = NMC // NCORES           # 11 chunks computed per core (P1 sharded)
SPLIT_WAITS = True              # walrus 1-wait/inst workaround

# fp8 DoubleRow enables. Measured rel-err (numpy sim, gate 2e-2): all-bf16
# 5.4e-3; whe 1.6e-2; scatter 1.7e-2; u 4.5e-2; v 4.6e-2 -> fp8 is too
# lossy on every site, keep everything bf16.
FP8_WHE = False     # h @ W_he
FP8_SCATTER = False  # S@h / S@hid2
FP8_U = False       # agg @ U1 / U2
FP8_V = False       # hid3 @ V
DEBUG_DUMP = False  # dump sb=0 intermediates to DRAM (sim debugging)
USE_DMAT = True     # XBAR DMA-transpose for zT (else 4 extra PE matmuls)
USE_ACTBIAS = True  # bias operand on scalar activation (h4 + b4)
SHARD_P1 = True     # shard mesh_proj across cores + AllGather
TRUNC = 99          # debug: truncate program after phase N
SKIP_LOADS = ()     # debug: skip superblock loads by tag


# ---------------------------------------------------------------- tile patch
def _patched_drain_and_barrier(self, tick_clock, wait_clock):
    # This walrus build accepts at most 1 sync wait per instruction; the
    # stock tail drain carries one wait per active proc.  Emit explicit
    # wait_ge instructions instead.
    probe = self.nc.sync.nop()
    if probe.ins.sync_info is None:
        probe.ins.sync_info = mybir.SyncInfo(on_wait=[], on_update=[])
    wait_clock.add_sem_waits(probe.ins, ScopedClock({None: tick_clock.global_clock}))
    waits = list(probe.ins.sync_info.on_wait)
    del probe.ins.sync_info.on_wait[:]
    name2sem = {s.name: s for s in self.sems.allocated().values()}
    for w in waits:
        self.nc.sync.wait_ge(name2sem[w.ant_name], w.wait_value)
    self.nc.sync.drain()
    self.nc.all_engine_barrier()
    assert self.sems is not None
    popped = self.nc._tile_sem_poison_stack.pop()
    assert popped is self._sem_poison
    self.nc.clear_and_free_semaphores(list(self.sems.allocated().values()))
    self.nc.all_engine_barrier()


tile.TileContext._drain_and_barrier = _patched_drain_and_barrier


# ------------------------------------------------------------------- helpers
def _wrap_idx(idx: np.ndarray) -> np.ndarray:
    """dma_gather index layout: index i at [i % 16, i // 16], the 16-row
    block replicated down all 128 partitions."""
    assert idx.size % 16 == 0
    w = idx.astype(np.int16).reshape(-1, 16).T  # [16, n/16]
    return np.ascontiguousarray(np.tile(w, (8, 1)))


def _chunkT(x: np.ndarray) -> np.ndarray:
    """[R,512] row-major -> [128, R/128 * 512] PE-ready transposed layout:
    out[p, (c*4+k)*128 + r] = x[c*128+r, k*128+p]."""
    R = x.shape[0]
    t = x.reshape(R // 128, 128, 4, 128).transpose(3, 0, 2, 1)
    return np.ascontiguousarray(t.reshape(128, R * 4))


# ------------------------------------------------------------- bass builder
def build_bass(CAPP):
    """Per-core Bass program (shared by all 8 cores). CAPP = even number of
    128-edge chunks per 128-dst block."""
    NCH = NB * CAPP              # chunks per core
    ECP = NCH * 128              # edge slots per core
    W8 = FP8 if FP8_WHE else BF16
    S8 = FP8 if FP8_SCATTER else BF16
    U8 = FP8 if FP8_U else BF16
    V8 = FP8 if FP8_V else BF16

    nc = bass.Bass("TRN2", target_bir_lowering=False, debug=False,
                   num_devices=NCORES)

    def din(name, shape, dt):
        return nc.dram_tensor(name, shape, dt, kind="ExternalInput").ap()

    meshTc = din("meshTc", [128, (NMC_L if SHARD_P1 else NMC) * 512], BF16)
    gridTc = din("gridTc", [128, NB * 512], BF16)
    attrsT5 = din("attrsT5", [5, ECP], BF16)
    srcidx = din("srcidx", [128, ECP // 16], I16)
    sel_d = din("sel", [128, NCH, 128], FP8)     # [e, chunk, d]
    selT_d = din("selT", [128, NCH, 128], FP8)   # [d, chunk, e]
    cntones = din("cntones", [2, NGS], BF16)
    w_ws = din("w_ws", [D, D], BF16)
    w_wd = din("w_wd", [D, D], BF16)
    w_whe = din("w_whe", [D, D], W8)
    w_emb0 = din("w_emb0", [5, D], BF16)
    w_u1 = din("w_u1", [D, D], U8)
    w_u2 = din("w_u2", [D, D], U8)
    w_w0a = din("w_w0a", [D, D], BF16)
    w_ow0 = din("w_ow0", [D, D], BF16)
    w_v = din("w_v", [D, D], V8)
    w_ow1 = din("w_ow1", [D, OUTD], BF16)
    v3b3 = din("v3b3", [2, D], BF16)
    b2bc = din("b2bc", [128, D], BF16)
    b4col = din("b4col", [128, 4], F32)
    ob1bc = din("ob1bc", [128, OUTD], F32)

    outt = nc.dram_tensor("outt", [NGS, OUTD], F32, kind="ExternalOutput").ap()
    dbg = {}
    if DEBUG_DUMP:
        for nm, shape in [("d_srcG", [128, D]), ("d_zb", [128, D]),
                          ("d_zT", [128, 4 * 128]), ("d_pre2s", [128, D]),
                          ("d_aggHT", [128, 4 * 4 * 128]),
                          ("d_aggIT", [128, 4 * 4 * 128]),
                          ("d_h3", [128, 4 * D]), ("d_h4", [128, 4 * D]),
                          ("d_gTb", [128, 4 * 4 * 128])]:
            dbg[nm] = nc.dram_tensor(nm, shape, BF16,
                                     kind="ExternalOutput").ap()

    with tile.TileContext(nc) as tc:
        with tc.tile_pool(name="const", bufs=1) as cp, \
             tc.tile_pool(name="dram", bufs=1, space="DRAM") as dp, \
             tc.tile_pool(name="io", bufs=2) as io, \
             tc.tile_pool(name="io3", bufs=3) as io3, \
             tc.tile_pool(name="work", bufs=3) as wk, \
             tc.tile_pool(name="workbig", bufs=2) as wkb, \
             tc.tile_pool(name="psA", bufs=4, space="PSUM") as psA, \
             tc.tile_pool(name="psAgg", bufs=2, space="PSUM") as psAgg:

            nc.gpsimd.load_library(library_config.mlp)
            rSB = nc.gpsimd.to_reg(CAPP * 128)

            # ---- DRAM scratch
            meshproj = dp.tile([NM, D], BF16)
            if SHARD_P1:
                meshproj_l = dp.tile([NM // NCORES, D], BF16, tag="meshproj_l")
            else:
                meshproj_l = None
            gridproj = dp.tile([NGS, D], BF16)

            # ---- resident constants in SBUF
            def cload(ap, shape, dt, tag):
                t = cp.tile(shape, dt, tag=tag)
                nc.sync.dma_start(t[:], ap)
                return t

            def wload(ap, tag, dt=BF16, free=D):
                t = cp.tile([128, 4, free], dt, tag=tag)
                nc.sync.dma_start(
                    t[:], ap.rearrange("(k p) f -> p k f", p=128))
                return t

            ws_sb = wload(w_ws, "ws")
            wd_sb = wload(w_wd, "wd")
            whe_sb = wload(w_whe, "whe", dt=W8)
            u1_sb = wload(w_u1, "u1", dt=U8)
            u2_sb = wload(w_u2, "u2", dt=U8)
            w0a_sb = wload(w_w0a, "w0a")
            ow0_sb = wload(w_ow0, "ow0")
            v_sb = wload(w_v, "v", dt=V8)
            ow1_sb = wload(w_ow1, "ow1", free=OUTD)
            emb0_sb = cload(w_emb0, [5, D], BF16, "emb0")
            v3b3_sb = cload(v3b3, [2, D], BF16, "v3b3")
            b2bc_sb = cload(b2bc, [128, D], BF16, "b2bc")
            b4c_sb = cload(b4col, [128, 4], F32, "b4c")
            ob1_sb = cload(ob1bc, [128, OUTD], F32, "ob1")
            srci_sb = cload(srcidx, [128, ECP // 16], I16, "srci")

            # ---- P1: mesh_proj = mesh @ Ws  (4-chunk batched DMA).
            # With SHARD_P1 each core computes NMC_L chunks, AllGather joins.
            P1C = NMC_L if SHARD_P1 else NMC
            p1dst = meshproj_l if SHARD_P1 else meshproj
            for c4 in range((P1C + 3) // 4):
                cn = min(4, P1C - c4 * 4)
                mT = io3.tile([128, 16, 128], BF16, tag="p1g")
                nc.sync.dma_start(
                    mT[:, :cn * 4, :],
                    meshTc[:, c4 * 2048:c4 * 2048 + cn * 512]
                    .rearrange("p (k r) -> p k r", r=128))
                for cc in range(cn):
                    c = c4 * 4 + cc
                    ps = psA.tile([128, D], F32, tag="mm")
                    for k in range(4):
                        nc.tensor.matmul(ps[:], mT[:, cc * 4 + k, :],
                                         ws_sb[:, k, :],
                                         start=(k == 0), stop=(k == 3))
                    mp = io3.tile([128, D], BF16, tag="p1o")
                    nc.vector.tensor_copy(mp[:], ps[:])
                    nc.sync.dma_start(p1dst[c * 128:(c + 1) * 128, :],
                                      mp[:])
            if SHARD_P1:
                nc.gpsimd.collective_compute(
                    "AllGather", mybir.AluOpType.bypass,
                    replica_groups=[list(range(NCORES))],
                    ins=[meshproj_l[:]], outs=[meshproj[:]])

            # ---- P2: grid_proj = grid @ Wd + b2 -> DRAM (4-chunk batched)
            for c4 in range(NB // 4 if TRUNC >= 2 else 0):
                gT = io3.tile([128, 16, 128], BF16, tag="p2g")
                nc.sync.dma_start(
                    gT[:], gridTc[:, c4 * 2048:(c4 + 1) * 2048]
                    .rearrange("p (k r) -> p k r", r=128))
                for cc in range(4):
                    c = c4 * 4 + cc
                    ps = psA.tile([128, D], F32, tag="mm")
                    for k in range(4):
                        nc.tensor.matmul(ps[:], gT[:, cc * 4 + k, :],
                                         wd_sb[:, k, :],
                                         start=(k == 0), stop=(k == 3))
                    gp = io3.tile([128, D], BF16, tag="p1o")
                    nc.vector.tensor_add(gp[:], ps[:], b2bc_sb[:])
                    nc.sync.dma_start(gridproj[c * 128:(c + 1) * 128, :],
                                      gp[:])

            # ---- fused edge + node phase, per 512-row superblock
            NPAIR = CAPP // 2
            for sb in range(NSB if TRUNC >= 3 else 0):
                c0 = sb * 4 * CAPP           # first chunk of superblock
                attrs_sb = io.tile([5, 4 * CAPP * 128], BF16, tag="attrs")
                if "attrs" not in SKIP_LOADS:
                    nc.sync.dma_start(
                        attrs_sb[:], attrsT5[:, c0 * 128:(c0 + 4 * CAPP) * 128])
                sel_sb = io.tile([128, 4 * CAPP, 128], FP8, tag="sel")
                if "sel" not in SKIP_LOADS:
                    nc.sync.dma_start(sel_sb[:], sel_d[:, c0:c0 + 4 * CAPP, :])
                selT_sb = io.tile([128, 4 * CAPP, 128], FP8, tag="selT")
                if "selT" not in SKIP_LOADS:
                    nc.sync.dma_start(selT_sb[:], selT_d[:, c0:c0 + 4 * CAPP, :])
                srcG = io.tile([128, 4 * CAPP, D], BF16, tag="srcG")
                if "srcG" not in SKIP_LOADS:
                    for bi in range(4):
                        o = c0 + bi * CAPP
                        nc.gpsimd.dma_gather(
                            srcG[:, bi * CAPP:(bi + 1) * CAPP, :],
                            meshproj[:],
                            srci_sb[:, o * 8:(o + CAPP) * 8],
                            num_idxs=CAPP * 128, num_idxs_reg=rSB,
                            elem_size=D)
                gTb = io.tile([128, 4, 4, 128], BF16, tag="gTb")
                if "gTb" not in SKIP_LOADS:
                    nc.sync.dma_start(
                        gTb[:], gridTc[:, sb * 2048:(sb + 1) * 2048]
                        .rearrange("p (c k r) -> p c k r", c=4, k=4))
                cnt_sb = io.tile([2, 512], BF16, tag="cnt")
                if "cnt" not in SKIP_LOADS:
                    nc.sync.dma_start(cnt_sb[:],
                                      cntones[:, sb * 512:(sb + 1) * 512])

                aggHT = wkb.tile([128, 4, 4, 128], U8, tag="aggHT")
                aggIT = wkb.tile([128, 4, 4, 128], U8, tag="aggIT")

                for bi in range(4):
                    b = sb * 4 + bi
                    gp_b = io3.tile([128, D], BF16, tag="gp_b")
                    nc.sync.dma_start(gp_b[:],
                                      gridproj[b * 128:(b + 1) * 128, :])
                    aggH_ps = psAgg.tile([128, 4, 128], F32, tag="aggH")
                    aggI_ps = psAgg.tile([128, 4, 128], F32, tag="aggI")

                    for ci in range(CAPP if TRUNC >= 4 else 0):
                        c = bi * CAPP + ci   # chunk within superblock
                        half = ci % 2
                        # z = attrs @ emb_w0  (edge-major)
                        psz = psA.tile([128, D], F32, tag="mm")
                        nc.tensor.matmul(
                            psz[:], attrs_sb[:, c * 128:(c + 1) * 128],
                            emb0_sb[:], start=True, stop=True)
                        if half == 0:
                            hRp = wk.tile([128, 2, D], S8, tag="hRp")
                            hid2p = wk.tile([128, 2, D], S8, tag="hid2p")
                        nc.scalar.activation(hRp[:, half, :], psz[:], AF.Silu)
                        hF = wk.tile([128, 4, 128], W8, tag="hF")
                        if USE_DMAT:
                            # zT via XBAR DMA transpose, then silu -> hF
                            zb = wk.tile([128, D], BF16, tag="zb")
                            nc.scalar.copy(zb[:], psz[:])
                            zT = wk.tile([128, 4, 128], BF16, tag="zT")
                            nc.sync.dma_start(zT[:], zb[:], transpose=True)
                            nc.scalar.activation(hF[:], zT[:], AF.Silu)
                        else:
                            # zT = emb_w0^T-chunks @ attrs directly on PE
                            psF = psA.tile([128, 4, 128], F32, tag="mm")
                            for k in range(4):
                                nc.tensor.matmul(
                                    psF[:, k, :],
                                    emb0_sb[:, k * 128:(k + 1) * 128],
                                    attrs_sb[:, c * 128:(c + 1) * 128],
                                    start=(k == 0), stop=(k == 3),
                                    skip_group_check=True)
                            nc.scalar.activation(hF[:], psF[:], AF.Silu)
                        # pre2 = gp[dst] + h @ W_he  (PSUM accum)
                        psP = psA.tile([128, D], F32, tag="mm")
                        nc.tensor.matmul(psP[:], selT_sb[:, c, :], gp_b[:],
                                         start=True, stop=False,
                                         skip_group_check=True)
                        if FP8_WHE:
                            for j in range(2):
                                nc.tensor.matmul(
                                    psP[:], hF[:, 2 * j:2 * j + 2, :],
                                    whe_sb[:, 2 * j:2 * j + 2, :],
                                    perf_mode=DR, start=False, stop=(j == 1),
                                    skip_group_check=True)
                        else:
                            for k in range(4):
                                nc.tensor.matmul(
                                    psP[:], hF[:, k, :], whe_sb[:, k, :],
                                    start=False, stop=(k == 3),
                                    skip_group_check=True)
                        pre2s = wk.tile([128, D], BF16, tag="pre2s")
                        nc.vector.tensor_add(pre2s[:], psP[:],
                                             srcG[:, c, :])
                        nc.scalar.activation(hid2p[:, half, :], pre2s[:],
                                             AF.Silu)
                        if DEBUG_DUMP and sb == 0 and c == 0:
                            nc.sync.dma_start(dbg["d_srcG"], srcG[:, 0, :])
                            nc.sync.dma_start(dbg["d_zb"], zb[:])
                            nc.sync.dma_start(
                                dbg["d_zT"],
                                zT[:].rearrange("p a b -> p (a b)"))
                            nc.sync.dma_start(dbg["d_pre2s"], pre2s[:])
                        # feature-major scatter into agg PSUM
                        # feature-major scatter (agg comes out transposed,
                        # ready for the node MLP). start=True zeroes the
                        # whole 2KB PSUM bank -> only (ci==0, k==0).
                        if TRUNC >= 5:
                            for k in range(4):
                                nc.tensor.matmul(
                                    aggH_ps[:, k, :],
                                    hRp[:, half, k * 128:(k + 1) * 128],
                                    sel_sb[:, c, :],
                                    start=(ci == 0 and k == 0),
                                    stop=(ci == CAPP - 1 and k == 3),
                                    skip_group_check=True)
                            for k in range(4):
                                nc.tensor.matmul(
                                    aggI_ps[:, k, :],
                                    hid2p[:, half, k * 128:(k + 1) * 128],
                                    sel_sb[:, c, :],
                                    start=(ci == 0 and k == 0),
                                    stop=(ci == CAPP - 1 and k == 3),
                                    skip_group_check=True)

                    if TRUNC >= 5:
                        nc.scalar.copy(aggHT[:, :, bi, :], aggH_ps[:])
                        nc.scalar.copy(aggIT[:, :, bi, :], aggI_ps[:])
                if DEBUG_DUMP and sb == 0:
                    nc.sync.dma_start(
                        dbg["d_aggHT"],
                        aggHT[:].rearrange("p a b c -> p (a b c)"))
                    nc.sync.dma_start(
                        dbg["d_aggIT"],
                        aggIT[:].rearrange("p a b c -> p (a b c)"))
                    nc.sync.dma_start(
                        dbg["d_gTb"],
                        gTb[:].rearrange("p a b c -> p (a b c)"))

                # ---- node MLP hidden (feature-major over 512 rows)
                h3 = wkb.tile([128, 4, D], V8, tag="h3")
                for g in range(4 if TRUNC >= 6 else 0):
                    gs = slice(g * 128, (g + 1) * 128)
                    ps3 = psA.tile([128, D], F32, tag="mm")
                    for k in range(4):
                        nc.tensor.matmul(ps3[:], w0a_sb[:, k, gs],
                                         gTb[:, :, k, :],
                                         start=(k == 0), stop=False)
                    if FP8_U:
                        for j in range(2):
                            jj = slice(2 * j, 2 * j + 2)
                            nc.tensor.matmul(ps3[:], u1_sb[:, jj, gs],
                                             aggHT[:, jj, :, :],
                                             perf_mode=DR,
                                             start=False, stop=False)
                        for j in range(2):
                            jj = slice(2 * j, 2 * j + 2)
                            nc.tensor.matmul(ps3[:], u2_sb[:, jj, gs],
                                             aggIT[:, jj, :, :],
                                             perf_mode=DR,
                                             start=False, stop=False)
                    else:
                        for k in range(4):
                            nc.tensor.matmul(ps3[:], u1_sb[:, k, gs],
                                             aggHT[:, k, :, :],
                                             start=False, stop=False)
                        for k in range(4):
                            nc.tensor.matmul(ps3[:], u2_sb[:, k, gs],
                                             aggIT[:, k, :, :],
                                             start=False, stop=False)
                    nc.tensor.matmul(ps3[:], v3b3_sb[:, gs], cnt_sb[:],
                                     start=False, stop=True)
                    nc.scalar.activation(h3[:, g, :], ps3[:], AF.Silu)

                # ---- pre4 = grid@ow0 + h3@V + b4 ; h4 = silu(pre4)
                h4 = wkb.tile([128, 4, D], BF16, tag="h4")
                for g in range(4 if TRUNC >= 7 else 0):
                    gs = slice(g * 128, (g + 1) * 128)
                    ps4 = psA.tile([128, D], F32, tag="mm")
                    for k in range(4):
                        nc.tensor.matmul(ps4[:], ow0_sb[:, k, gs],
                                         gTb[:, :, k, :],
                                         start=(k == 0), stop=False)
                    if FP8_V:
                        for j in range(2):
                            jj = slice(2 * j, 2 * j + 2)
                            nc.tensor.matmul(ps4[:], v_sb[:, jj, gs],
                                             h3[:, jj, :], perf_mode=DR,
                                             start=False,
                                             stop=(j == 1))
                    else:
                        for k in range(4):
                            nc.tensor.matmul(ps4[:], v_sb[:, k, gs],
                                             h3[:, k, :],
                                             start=False, stop=(k == 3))
                    if USE_ACTBIAS:
                        nc.scalar.activation(h4[:, g, :], ps4[:], AF.Silu,
                                             bias=b4c_sb[:, g:g + 1])
                    else:
                        t4 = wk.tile([128, D], BF16, tag="t4")
                        nc.vector.tensor_scalar(t4[:], ps4[:],
                                                b4c_sb[:, g:g + 1], None,
                                                op0=ALU.add)
                        nc.scalar.activation(h4[:, g, :], t4[:], AF.Silu)
                if DEBUG_DUMP and sb == 0:
                    nc.sync.dma_start(
                        dbg["d_h3"], h3[:].rearrange("p a b -> p (a b)"))
                    nc.sync.dma_start(
                        dbg["d_h4"], h4[:].rearrange("p a b -> p (a b)"))

                # ---- out = h4 @ ow1 + ob1  (row-major out)
                for sc in range(4 if TRUNC >= 7 else 0):
                    rs = slice(sc * 128, (sc + 1) * 128)
                    pso = psA.tile([128, OUTD], F32, tag="mm")
                    for g in range(4):
                        nc.tensor.matmul(pso[:], h4[:, g, rs],
                                         ow1_sb[:, g, :],
                                         start=(g == 0), stop=(g == 3))
                    ot = io3.tile([128, OUTD], F32, tag="ot")
                    nc.vector.tensor_add(ot[:], pso[:], ob1_sb[:])
                    nc.sync.dma_start(
                        outt[sb * 512 + sc * 128:sb * 512 + (sc + 1) * 128, :],
                        ot[:])

    from concourse.library_overlay import lower_extended_insts
    lower_extended_insts(nc)
    if SPLIT_WAITS:
        _split_multi_waits(nc)
    return nc


def _split_multi_waits(nc):
    """This walrus build allows at most ONE sync wait per instruction.
    Move surplus waits onto EventSemaphore carrier instructions inserted
    immediately before, on the same engine."""
    for f in nc.m.functions:
        for bb in f.blocks:
            insts = list(bb.instructions)
            if not any(i.sync_info is not None and len(i.sync_info.on_wait) > 1
                       for i in insts):
                continue
            new = []
            for ins in insts:
                si = ins.sync_info
                if si is not None and len(si.on_wait) > 1:
                    waits = list(si.on_wait)
                    for w in waits[:-1]:
                        c = mybir.InstEventSemaphore(
                            name=f"I-w{nc.next_id()}", engine=ins.engine,
                            ins=[], outs=[],
                            sync_info=mybir.SyncInfo(on_wait=[w], on_update=[]))
                        new.append(c)
                    del si.on_wait[:]
                    si.on_wait.append(waits[-1])
                new.append(ins)
            bb.instructions = new


# ------------------------------------------------------------ host pipeline
def _pack_rows(deg, nbins, cap_edges):
    """Assign nbins*128 rows (with edge degrees `deg`) to nbins bins of
    exactly 128 rows such that each bin's total degree <= cap_edges.
    Greedy: rows by descending degree into the min-load bin with row room.
    Returns the bin assignment or None if the cap is infeasible."""
    import heapq
    order = np.argsort(-deg, kind="stable")
    assign = np.empty(nbins * 128, np.int64)
    heap = [(0, 0, b) for b in range(nbins)]
    heapq.heapify(heap)
    for r in order:
        while True:
            e, rr, b = heapq.heappop(heap)
            if rr < 128:
                break
        if e + deg[r] > cap_edges:
            return None
        assign[r] = b
        heapq.heappush(heap, (e + int(deg[r]), rr + 1, b))
    return assign


def _prep(inputs):
    """Host-side weight folding + edge packing. Returns (in_maps, CAPP)."""
    mesh_f = np.asarray(inputs["mesh_node_features"])[0]   # [N_MESH, D]
    grid_f = np.asarray(inputs["grid_node_features"])[0]   # [N_GRID, D]
    attrs = np.asarray(inputs["edge_attrs"])               # [E, 4]
    esrc = np.asarray(inputs["edge_src"]).astype(np.int64)
    edst = np.asarray(inputs["edge_dst"]).astype(np.int64)

    # ---- fold weights (fp32 on host)
    W = {k: np.asarray(inputs[k], np.float32) for k in (
        "emb_w0", "emb_b0", "emb_w1", "emb_b1", "edge_w0", "edge_b0",
        "edge_w1", "edge_b1", "node_w0", "node_b0", "node_w1", "node_b1",
        "out_w0", "out_b0", "out_w1", "out_b1")}
    Ws, Wd, We = W["edge_w0"][:D], W["edge_w0"][D:2 * D], W["edge_w0"][2 * D:]
    W0a, W0b = W["node_w0"][:D], W["node_w0"][D:]
    W_he = W["emb_w1"] @ We
    b2 = W["emb_b1"] @ We + W["edge_b0"]
    U1 = W["emb_w1"] @ W0b
    U2 = W["edge_w1"] @ W0b
    v3 = (W["emb_b1"] + W["edge_b1"]) @ W0b
    V = W["node_w1"] @ W["out_w0"]
    b4 = W["node_b1"] @ W["out_w0"] + W["out_b0"]
    emb_w0b = np.concatenate([W["emb_w0"], W["emb_b0"][None]], 0)  # [5, D]
    v3b3 = np.stack([v3, W["node_b0"]], 0)                          # [2, D]

    # ---- global row -> (core, block) bin packing. Rows are freely
    # permutable (host unpermutes the output), so balance shard load and
    # per-block edge caps in one 512-bin pack.
    NBINS = NCORES * NB
    NRT = NBINS * 128                     # 65536 row slots
    deg = np.bincount(edst, minlength=NRT)
    CAP = max(2, int(math.ceil(len(edst) / (NBINS * 128.0))))
    while True:
        assign = _pack_rows(deg, NBINS, CAP * 128)
        if assign is not None:
            break
        CAP += 1
    CAPP = CAP + (CAP % 2) if FP8_SCATTER else CAP
    NCH = NB * CAPP
    ECP = NCH * 128
    # global packed position of each row; bins are filled to exactly 128
    order_by_bin = np.argsort(assign, kind="stable")
    pos_global = np.empty(NRT, np.int64)
    pos_global[order_by_bin] = np.arange(NRT)
    nd_global = pos_global[edst]
    core_of = nd_global // NGS

    w8 = f8 if FP8_WHE else bf
    u8 = f8 if FP8_U else bf
    v8 = f8 if FP8_V else bf
    meshTc_full = _chunkT(np.concatenate(
        [mesh_f, np.zeros((NM - N_MESH, D), np.float32)]).astype(bf))
    shared = {
        "w_ws": Ws.astype(bf), "w_wd": Wd.astype(bf),
        "w_whe": W_he.astype(w8), "w_emb0": emb_w0b.astype(bf),
        "w_u1": U1.astype(u8), "w_u2": U2.astype(u8),
        "w_w0a": W0a.astype(bf), "w_ow0": W["out_w0"].astype(bf),
        "w_v": V.astype(v8), "w_ow1": W["out_w1"].astype(bf),
        "v3b3": v3b3.astype(bf),
        "b2bc": np.ascontiguousarray(
            np.broadcast_to(b2[None], (128, D))).astype(bf),
        "b4col": np.ascontiguousarray(
            b4.reshape(4, 128).T).astype(np.float32),
        "ob1bc": np.ascontiguousarray(
            np.broadcast_to(W["out_b1"][None], (128, OUTD))).astype(
                np.float32),
    }

    grid_ext = np.zeros((NRT, D), np.float32)
    grid_ext[:N_GRID] = grid_f

    in_maps = []
    for core in range(NCORES):
        m = core_of == core
        cs, ca = esrc[m], attrs[m]
        nd = nd_global[m] - core * NGS       # local packed destination
        eo = np.argsort(nd, kind="stable")   # group edges by block
        cs, nd, ca = cs[eo], nd[eo], ca[eo]
        cb = nd // 128
        src_p = np.zeros(ECP, np.int16)
        att_p = np.zeros((ECP, 4), np.float32)
        nbc = np.bincount(cb, minlength=NB)
        assert nbc.max() <= CAPP * 128
        starts = np.arange(NB) * CAPP * 128
        pos_in_blk = np.arange(len(cs)) - np.repeat(
            np.cumsum(nbc) - nbc, nbc)
        slot = starts[cb] + pos_in_blk
        src_p[slot] = cs
        att_p[slot] = ca
        attrsT5 = np.concatenate(
            [att_p.T, np.ones((1, ECP), np.float32)], 0).astype(bf)
        e_lo = slot % 128
        ch = slot // 128
        d_lo = nd - cb * 128
        sel = np.zeros((128, NCH, 128), f8)
        sel[e_lo, ch, d_lo] = 1.0
        selT = np.zeros((128, NCH, 128), f8)
        selT[d_lo, ch, e_lo] = 1.0
        grid_perm = grid_ext[order_by_bin[core * NGS:(core + 1) * NGS]]
        cnt = np.zeros(NGS, np.float32)
        np.add.at(cnt, nd, 1.0)
        cntones = np.stack([cnt, np.ones(NGS, np.float32)], 0).astype(bf)
        mtc = (meshTc_full[:, core * NMC_L * 512:(core + 1) * NMC_L * 512]
               if SHARD_P1 else meshTc_full)
        in_maps.append(dict(shared,
                            meshTc=np.ascontiguousarray(mtc),
                            gridTc=_chunkT(grid_perm.astype(bf)),
                            attrsT5=np.ascontiguousarray(attrsT5),
                            srcidx=_wrap_idx(src_p),
                            sel=sel, selT=selT,
                            cntones=cntones,
                            _pos_global=pos_global))
    return in_maps, CAPP


_CACHE = {}


class _Runner:
    """Persistent jitted SPMD executor (avoids re-jitting per call)."""

    def __init__(self, nc):
        import jax
        from jax.experimental.shard_map import shard_map
        from jax.sharding import Mesh, PartitionSpec
        from concourse import bass2jax

        bass2jax.install_neuronx_cc_hook()
        self.nc = nc
        part_name = (nc.partition_id_tensor.name
                     if nc.partition_id_tensor else None)
        in_names, out_names, out_avals, zero_outs = [], [], [], []
        for alloc in nc.m.functions[0].allocations:
            if not isinstance(alloc, mybir.MemoryLocationSet):
                continue
            name = alloc.memorylocations[0].name
            if alloc.kind == "ExternalInput":
                if name != part_name:
                    in_names.append(name)
            elif alloc.kind == "ExternalOutput":
                shape = tuple(alloc.tensor_shape)
                dtype = mybir.dt.np(alloc.dtype)
                out_names.append(name)
                out_avals.append(jax.core.ShapedArray(shape, dtype))
                zero_outs.append(np.zeros(shape, dtype))
        self.in_names = list(in_names)
        self.out_names = out_names
        self.out_shapes = [tuple(a.shape) for a in out_avals]
        all_names = in_names + out_names
        if part_name is not None:
            all_names = all_names + [part_name]

        def _body(*args):
            operands = list(args)
            if part_name is not None:
                operands.append(bass2jax.partition_id_tensor())
            outs = bass2jax._bass_exec_p.bind(
                *operands,
                out_avals=tuple(out_avals),
                in_names=tuple(all_names),
                out_names=tuple(out_names),
                lowering_input_output_aliases=(),
                sim_require_finite=True,
                sim_require_nnan=True,
                nc=nc,
            )
            return tuple(outs)

        devices = jax.devices()[:NCORES]
        mesh = Mesh(np.asarray(devices), ("core",))
        nin = len(self.in_names) + len(out_names)
        self.fn = jax.jit(shard_map(
            _body, mesh=mesh,
            in_specs=(PartitionSpec("core"),) * nin,
            out_specs=(PartitionSpec("core"),) * len(out_names),
            check_rep=False))
        self.zero_outs = zero_outs
        self.sharding = jax.sharding.NamedSharding(mesh, PartitionSpec("core"))
        self.mesh = mesh
        self._avals = out_avals
        self._jax = jax

    def put(self, in_maps):
        arrs = []
        for name in self.in_names:
            arrs.append(np.concatenate([m[name] for m in in_maps], axis=0))
        for z in self.zero_outs:
            arrs.append(np.concatenate([z] * NCORES, axis=0))
        return [self._jax.device_put(a, self.sharding) for a in arrs]

    def run(self, arrs):
        return self.fn(*arrs)

    def get(self, outs):
        res = [np.asarray(o) for o in outs]
        per_core = []
        for c in range(NCORES):
            d = {}
            for i, name in enumerate(self.out_names):
                n0 = self.out_shapes[i][0]
                d[name] = res[i][c * n0:(c + 1) * n0]
            per_core.append(d)
        return per_core


def _get_runner(CAPP) -> _Runner:
    if CAPP not in _CACHE:
        _CACHE[CAPP] = _Runner(build_bass(CAPP))
    return _CACHE[CAPP]


def kernel(**inputs) -> np.ndarray:
    in_maps, CAPP = _prep(inputs)
    r = _get_runner(CAPP)
    outs = r.run(r.put(in_maps))
    per_core = r.get(outs)
    # rows were bin-packed into (core, block) bins on device; unpermute
    big = np.concatenate([per_core[c]["outt"] for c in range(NCORES)], axis=0)
    out = big[in_maps[0]["_pos_global"][:N_GRID]]
    return out[None].astype(np.float32)


# revision 50
# speedup vs baseline: 1.0104x; 1.0104x over previous
"""Trainium2 Bass kernel for Mesh2GridDecoder (GraphCast-style mesh->grid
message passing + output MLP), distributed over 8 NeuronCores.

Sharding: grid nodes (and edges, by destination) sharded across 8 cores so
the scatter-sum is core-local; mesh node features and weights replicated.

Math restructuring (exact, up to float re-association):
  h     = silu(attrs @ emb_w0 + emb_b0)                       per edge
  pre2  = mesh_proj[src] + grid_proj[dst] + h @ W_he
    with mesh_proj = mesh@Ws, grid_proj = grid@Wd + (emb_b1@We + edge_b0),
         W_he = emb_w1 @ We      (Ws|Wd|We = row blocks of edge_w0)
  hid2  = silu(pre2)
  pre3  = grid@W0a + (S@h)@U1 + (S@hid2)@U2 + cnt (x) v3 + node_b0
    with U1 = emb_w1@W0b, U2 = edge_w1@W0b, v3 = (emb_b1+edge_b1)@W0b
         (S = scatter-sum matrix, cnt = per-node edge count)
  hid3  = silu(pre3)
  pre4  = grid@out_w0 + hid3@V + b4,   V = node_w1@out_w0,
          b4 = node_b1@out_w0 + out_b0
  out   = silu(pre4) @ out_w1 + out_b1

Device structure per core:
  P1: mesh_proj = meshT @ Ws   (host pre-transposed mesh, plain DMA)
  P2: grid_proj = gridT @ Wd + b2 -> DRAM
  Fused edge+node phase per 512-row superblock (16 of them):
    per 128-dst block (4 per superblock), per 128-edge chunk:
      z  = attrs-chunk @ emb_w0  (PE, edge-major)
      hR = silu(z)               (scalar, fp8 out for the scatter)
      zT = XBAR DMA-transpose(z); hF = silu(zT)   (feature-major)
      pre2 = selT-mm(grid_proj block) + hF@W_he [DoubleRow fp8] in PSUM,
             + mesh_proj[src] (gpsimd gather) via one vector add
      hid2 = silu(pre2)          (scalar, fp8)
      scatter FEATURE-major: aggT[f,d] += hR/hid2 (lhsT) @ sel-chunk,
             fp8 DoubleRow over chunk pairs
    node+out MLPs on the 512 rows: weights feature-major, activations
    feature-major throughout; u1/u2/v matmuls in fp8 DoubleRow; biases
    folded into activations / the PSUM->SBUF copy.
"""
import math
import numpy as np
import ml_dtypes

import concourse.bass as bass
import concourse.tile as tile
from concourse import mybir
from concourse import bass_utils
from concourse import library_config
from concourse.vector_clock import ScopedClock

BF16 = mybir.dt.bfloat16
F32 = mybir.dt.float32
I16 = mybir.dt.int16
FP8 = mybir.dt.float8e4
AF = mybir.ActivationFunctionType
ALU = mybir.AluOpType
DR = mybir.MatmulPerfMode.DoubleRow
bf = ml_dtypes.bfloat16
f8 = ml_dtypes.float8_e4m3

N_MESH = 10242
N_GRID = 65160
N_EDGES = 195480
D = 512
OUTD = 471
NCORES = 8
GSH = N_GRID // NCORES          # 8145 grid rows per core
NGS = 8192                      # padded grid shard rows
NB = NGS // 128                 # 64 dst blocks per core
NSB = NB // 4                   # 16 superblocks (512 rows each)
NM = 11264                      # padded mesh rows (88 chunks of 128, /8 cores)
NMC = NM // 128
NMC_L = NMC // NCORES           # 11 chunks computed per core (P1 sharded)
SPLIT_WAITS = True              # walrus 1-wait/inst workaround

# fp8 DoubleRow enables. Measured rel-err (numpy sim, gate 2e-2): all-bf16
# 5.4e-3; whe 1.6e-2; scatter 1.7e-2; u 4.5e-2; v 4.6e-2 -> fp8 is too
# lossy on every site, keep everything bf16.
FP8_WHE = False     # h @ W_he
FP8_SCATTER = False  # S@h / S@hid2
FP8_U = False       # agg @ U1 / U2
FP8_V = False       # hid3 @ V
DEBUG_DUMP = False  # dump sb=0 intermediates to DRAM (sim debugging)
USE_DMAT = True     # XBAR DMA-transpose for zT (else 4 extra PE matmuls)
USE_ACTBIAS = True  # bias operand on scalar activation (h4 + b4)
SHARD_P1 = True     # shard mesh_proj across cores + AllGather
TRUNC = 99          # debug: truncate program after phase N
SKIP_LOADS = ()     # debug: skip superblock loads by tag


# ---------------------------------------------------------------- tile patch
def _patched_drain_and_barrier(self, tick_clock, wait_clock):
    # This walrus build accepts at most 1 sync wait per instruction; the
    # stock tail drain carries one wait per active proc.  Emit explicit
    # wait_ge instructions instead.
    probe = self.nc.sync.nop()
    if probe.ins.sync_info is None:
        probe.ins.sync_info = mybir.SyncInfo(on_wait=[], on_update=[])
    wait_clock.add_sem_waits(probe.ins, ScopedClock({None: tick_clock.global_clock}))
    waits = list(probe.ins.sync_info.on_wait)
    del probe.ins.sync_info.on_wait[:]
    name2sem = {s.name: s for s in self.sems.allocated().values()}
    for w in waits:
        self.nc.sync.wait_ge(name2sem[w.ant_name], w.wait_value)
    self.nc.sync.drain()
    self.nc.all_engine_barrier()
    assert self.sems is not None
    popped = self.nc._tile_sem_poison_stack.pop()
    assert popped is self._sem_poison
    self.nc.clear_and_free_semaphores(list(self.sems.allocated().values()))
    self.nc.all_engine_barrier()


tile.TileContext._drain_and_barrier = _patched_drain_and_barrier


# ------------------------------------------------------------------- helpers
def _wrap_idx(idx: np.ndarray) -> np.ndarray:
    """dma_gather index layout: index i at [i % 16, i // 16], the 16-row
    block replicated down all 128 partitions."""
    assert idx.size % 16 == 0
    w = idx.astype(np.int16).reshape(-1, 16).T  # [16, n/16]
    return np.ascontiguousarray(np.tile(w, (8, 1)))


def _chunkT(x: np.ndarray) -> np.ndarray:
    """[R,512] row-major -> [128, R/128 * 512] PE-ready transposed layout:
    out[p, (c*4+k)*128 + r] = x[c*128+r, k*128+p]."""
    R = x.shape[0]
    t = x.reshape(R // 128, 128, 4, 128).transpose(3, 0, 2, 1)
    return np.ascontiguousarray(t.reshape(128, R * 4))


# ------------------------------------------------------------- bass builder
def build_bass(CAPP):
    """Per-core Bass program (shared by all 8 cores). CAPP = even number of
    128-edge chunks per 128-dst block."""
    NCH = NB * CAPP              # chunks per core
    ECP = NCH * 128              # edge slots per core
    W8 = FP8 if FP8_WHE else BF16
    S8 = FP8 if FP8_SCATTER else BF16
    U8 = FP8 if FP8_U else BF16
    V8 = FP8 if FP8_V else BF16

    nc = bass.Bass("TRN2", target_bir_lowering=False, debug=False,
                   num_devices=NCORES)

    def din(name, shape, dt):
        return nc.dram_tensor(name, shape, dt, kind="ExternalInput").ap()

    meshTc = din("meshTc", [128, (NMC_L if SHARD_P1 else NMC) * 512], BF16)
    gridTc = din("gridTc", [128, NB * 512], BF16)
    attrsT5 = din("attrsT5", [5, ECP], BF16)
    srcidx = din("srcidx", [128, ECP // 16], I16)
    sel_d = din("sel", [128, NCH, 128], FP8)     # [e, chunk, d]
    selT_d = din("selT", [128, NCH, 128], FP8)   # [d, chunk, e]
    cntones = din("cntones", [2, NGS], BF16)
    w_ws = din("w_ws", [D, D], BF16)
    w_wd = din("w_wd", [D, D], BF16)
    w_whe = din("w_whe", [D, D], W8)
    w_emb0 = din("w_emb0", [5, D], BF16)
    w_u1 = din("w_u1", [D, D], U8)
    w_u2 = din("w_u2", [D, D], U8)
    w_w0a = din("w_w0a", [D, D], BF16)
    w_ow0 = din("w_ow0", [D, D], BF16)
    w_v = din("w_v", [D, D], V8)
    w_ow1 = din("w_ow1", [D, OUTD], BF16)
    v3b3 = din("v3b3", [2, D], BF16)
    b2bc = din("b2bc", [128, D], BF16)
    b4col = din("b4col", [128, 4], F32)
    ob1bc = din("ob1bc", [128, OUTD], F32)

    outt = nc.dram_tensor("outt", [NGS, OUTD], F32, kind="ExternalOutput").ap()
    dbg = {}
    if DEBUG_DUMP:
        for nm, shape in [("d_srcG", [128, D]), ("d_zb", [128, D]),
                          ("d_zT", [128, 4 * 128]), ("d_pre2s", [128, D]),
                          ("d_aggHT", [128, 4 * 4 * 128]),
                          ("d_aggIT", [128, 4 * 4 * 128]),
                          ("d_h3", [128, 4 * D]), ("d_h4", [128, 4 * D]),
                          ("d_gTb", [128, 4 * 4 * 128])]:
            dbg[nm] = nc.dram_tensor(nm, shape, BF16,
                                     kind="ExternalOutput").ap()

    with tile.TileContext(nc) as tc:
        with tc.tile_pool(name="const", bufs=1) as cp, \
             tc.tile_pool(name="dram", bufs=1, space="DRAM") as dp, \
             tc.tile_pool(name="io", bufs=2) as io, \
             tc.tile_pool(name="io3", bufs=3) as io3, \
             tc.tile_pool(name="work", bufs=3) as wk, \
             tc.tile_pool(name="workbig", bufs=2) as wkb, \
             tc.tile_pool(name="psA", bufs=4, space="PSUM") as psA, \
             tc.tile_pool(name="psAgg", bufs=2, space="PSUM") as psAgg:

            nc.gpsimd.load_library(library_config.mlp)
            rSB = nc.gpsimd.to_reg(CAPP * 128)

            # ---- DRAM scratch
            meshproj = dp.tile([NM, D], BF16)
            if SHARD_P1:
                meshproj_l = dp.tile([NM // NCORES, D], BF16, tag="meshproj_l")
            else:
                meshproj_l = None
            gridproj = dp.tile([NGS, D], BF16)

            # ---- resident constants in SBUF
            def cload(ap, shape, dt, tag):
                t = cp.tile(shape, dt, tag=tag)
                nc.sync.dma_start(t[:], ap)
                return t

            def wload(ap, tag, dt=BF16, free=D):
                t = cp.tile([128, 4, free], dt, tag=tag)
                nc.sync.dma_start(
                    t[:], ap.rearrange("(k p) f -> p k f", p=128))
                return t

            ws_sb = wload(w_ws, "ws")
            wd_sb = wload(w_wd, "wd")
            whe_sb = wload(w_whe, "whe", dt=W8)
            u1_sb = wload(w_u1, "u1", dt=U8)
            u2_sb = wload(w_u2, "u2", dt=U8)
            w0a_sb = wload(w_w0a, "w0a")
            ow0_sb = wload(w_ow0, "ow0")
            v_sb = wload(w_v, "v", dt=V8)
            ow1_sb = wload(w_ow1, "ow1", free=OUTD)
            emb0_sb = cload(w_emb0, [5, D], BF16, "emb0")
            v3b3_sb = cload(v3b3, [2, D], BF16, "v3b3")
            b2bc_sb = cload(b2bc, [128, D], BF16, "b2bc")
            b4c_sb = cload(b4col, [128, 4], F32, "b4c")
            ob1_sb = cload(ob1bc, [128, OUTD], F32, "ob1")
            srci_sb = cload(srcidx, [128, ECP // 16], I16, "srci")

            # ---- P1: mesh_proj = mesh @ Ws  (4-chunk batched DMA).
            # With SHARD_P1 each core computes NMC_L chunks, AllGather joins.
            P1C = NMC_L if SHARD_P1 else NMC
            p1dst = meshproj_l if SHARD_P1 else meshproj
            for c4 in range((P1C + 3) // 4):
                cn = min(4, P1C - c4 * 4)
                mT = io3.tile([128, 16, 128], BF16, tag="p1g")
                nc.sync.dma_start(
                    mT[:, :cn * 4, :],
                    meshTc[:, c4 * 2048:c4 * 2048 + cn * 512]
                    .rearrange("p (k r) -> p k r", r=128))
                for cc in range(cn):
                    c = c4 * 4 + cc
                    ps = psA.tile([128, D], F32, tag="mm")
                    for k in range(4):
                        nc.tensor.matmul(ps[:], mT[:, cc * 4 + k, :],
                                         ws_sb[:, k, :],
                                         start=(k == 0), stop=(k == 3))
                    mp = io3.tile([128, D], BF16, tag="p1o")
                    nc.vector.tensor_copy(mp[:], ps[:])
                    nc.sync.dma_start(p1dst[c * 128:(c + 1) * 128, :],
                                      mp[:])
            if SHARD_P1:
                nc.gpsimd.collective_compute(
                    "AllGather", mybir.AluOpType.bypass,
                    replica_groups=[list(range(NCORES))],
                    ins=[meshproj_l[:]], outs=[meshproj[:]])

            # ---- P2: grid_proj = grid @ Wd + b2 -> DRAM (4-chunk batched)
            for c4 in range(NB // 4 if TRUNC >= 2 else 0):
                gT = io3.tile([128, 16, 128], BF16, tag="p2g")
                nc.sync.dma_start(
                    gT[:], gridTc[:, c4 * 2048:(c4 + 1) * 2048]
                    .rearrange("p (k r) -> p k r", r=128))
                for cc in range(4):
                    c = c4 * 4 + cc
                    ps = psA.tile([128, D], F32, tag="mm")
                    for k in range(4):
                        nc.tensor.matmul(ps[:], gT[:, cc * 4 + k, :],
                                         wd_sb[:, k, :],
                                         start=(k == 0), stop=(k == 3))
                    gp = io3.tile([128, D], BF16, tag="p1o")
                    nc.vector.tensor_add(gp[:], ps[:], b2bc_sb[:])
                    nc.sync.dma_start(gridproj[c * 128:(c + 1) * 128, :],
                                      gp[:])

            # ---- fused edge + node phase, per 512-row superblock
            NPAIR = CAPP // 2
            for sb in range(NSB if TRUNC >= 3 else 0):
                c0 = sb * 4 * CAPP           # first chunk of superblock
                attrs_sb = io.tile([5, 4 * CAPP * 128], BF16, tag="attrs")
                if "attrs" not in SKIP_LOADS:
                    nc.sync.dma_start(
                        attrs_sb[:], attrsT5[:, c0 * 128:(c0 + 4 * CAPP) * 128])
                sel_sb = io.tile([128, 4 * CAPP, 128], FP8, tag="sel")
                if "sel" not in SKIP_LOADS:
                    nc.sync.dma_start(sel_sb[:], sel_d[:, c0:c0 + 4 * CAPP, :])
                selT_sb = io.tile([128, 4 * CAPP, 128], FP8, tag="selT")
                if "selT" not in SKIP_LOADS:
                    nc.sync.dma_start(selT_sb[:], selT_d[:, c0:c0 + 4 * CAPP, :])
                srcG = io.tile([128, 4 * CAPP, D], BF16, tag="srcG")
                if "srcG" not in SKIP_LOADS:
                    for bi in range(4):
                        o = c0 + bi * CAPP
                        nc.gpsimd.dma_gather(
                            srcG[:, bi * CAPP:(bi + 1) * CAPP, :],
                            meshproj[:],
                            srci_sb[:, o * 8:(o + CAPP) * 8],
                            num_idxs=CAPP * 128, num_idxs_reg=rSB,
                            elem_size=D)
                gTb = io.tile([128, 4, 4, 128], BF16, tag="gTb")
                if "gTb" not in SKIP_LOADS:
                    nc.sync.dma_start(
                        gTb[:], gridTc[:, sb * 2048:(sb + 1) * 2048]
                        .rearrange("p (c k r) -> p c k r", c=4, k=4))
                cnt_sb = io.tile([2, 512], BF16, tag="cnt")
                if "cnt" not in SKIP_LOADS:
                    nc.sync.dma_start(cnt_sb[:],
                                      cntones[:, sb * 512:(sb + 1) * 512])

                aggHT = wkb.tile([128, 4, 4, 128], U8, tag="aggHT")
                aggIT = wkb.tile([128, 4, 4, 128], U8, tag="aggIT")

                for bi in range(4):
                    b = sb * 4 + bi
                    gp_b = io3.tile([128, D], BF16, tag="gp_b")
                    nc.sync.dma_start(gp_b[:],
                                      gridproj[b * 128:(b + 1) * 128, :])
                    aggH_ps = psAgg.tile([128, 4, 128], F32, tag="aggH")
                    aggI_ps = psAgg.tile([128, 4, 128], F32, tag="aggI")

                    for ci in range(CAPP if TRUNC >= 4 else 0):
                        c = bi * CAPP + ci   # chunk within superblock
                        half = ci % 2
                        # z = attrs @ emb_w0  (edge-major)
                        psz = psA.tile([128, D], F32, tag="mm")
                        nc.tensor.matmul(
                            psz[:], attrs_sb[:, c * 128:(c + 1) * 128],
                            emb0_sb[:], start=True, stop=True)
                        if half == 0:
                            hRp = wk.tile([128, 2, D], S8, tag="hRp")
                            hid2p = wk.tile([128, 2, D], S8, tag="hid2p")
                        nc.scalar.activation(hRp[:, half, :], psz[:], AF.Silu)
                        hF = wk.tile([128, 4, 128], W8, tag="hF")
                        if USE_DMAT:
                            # zT via XBAR DMA transpose, then silu -> hF
                            zb = wk.tile([128, D], BF16, tag="zb")
                            nc.scalar.copy(zb[:], psz[:])
                            zT = wk.tile([128, 4, 128], BF16, tag="zT")
                            nc.sync.dma_start(zT[:], zb[:], transpose=True)
                            nc.scalar.activation(hF[:], zT[:], AF.Silu)
                        else:
                            # zT = emb_w0^T-chunks @ attrs directly on PE
                            psF = psA.tile([128, 4, 128], F32, tag="mm")
                            for k in range(4):
                                nc.tensor.matmul(
                                    psF[:, k, :],
                                    emb0_sb[:, k * 128:(k + 1) * 128],
                                    attrs_sb[:, c * 128:(c + 1) * 128],
                                    start=(k == 0), stop=(k == 3),
                                    skip_group_check=True)
                            nc.scalar.activation(hF[:], psF[:], AF.Silu)
                        # pre2 = gp[dst] + h @ W_he  (PSUM accum)
                        psP = psA.tile([128, D], F32, tag="mm")
                        nc.tensor.matmul(psP[:], selT_sb[:, c, :], gp_b[:],
                                         start=True, stop=False,
                                         skip_group_check=True)
                        if FP8_WHE:
                            for j in range(2):
                                nc.tensor.matmul(
                                    psP[:], hF[:, 2 * j:2 * j + 2, :],
                                    whe_sb[:, 2 * j:2 * j + 2, :],
                                    perf_mode=DR, start=False, stop=(j == 1),
                                    skip_group_check=True)
                        else:
                            for k in range(4):
                                nc.tensor.matmul(
                                    psP[:], hF[:, k, :], whe_sb[:, k, :],
                                    start=False, stop=(k == 3),
                                    skip_group_check=True)
                        pre2s = wk.tile([128, D], BF16, tag="pre2s")
                        nc.vector.tensor_add(pre2s[:], psP[:],
                                             srcG[:, c, :])
                        nc.scalar.activation(hid2p[:, half, :], pre2s[:],
                                             AF.Silu)
                        if DEBUG_DUMP and sb == 0 and c == 0:
                            nc.sync.dma_start(dbg["d_srcG"], srcG[:, 0, :])
                            nc.sync.dma_start(dbg["d_zb"], zb[:])
                            nc.sync.dma_start(
                                dbg["d_zT"],
                                zT[:].rearrange("p a b -> p (a b)"))
                            nc.sync.dma_start(dbg["d_pre2s"], pre2s[:])
                        # feature-major scatter into agg PSUM
                        # feature-major scatter (agg comes out transposed,
                        # ready for the node MLP). start=True zeroes the
                        # whole 2KB PSUM bank -> only (ci==0, k==0).
                        if TRUNC >= 5:
                            for k in range(4):
                                nc.tensor.matmul(
                                    aggH_ps[:, k, :],
                                    hRp[:, half, k * 128:(k + 1) * 128],
                                    sel_sb[:, c, :],
                                    start=(ci == 0 and k == 0),
                                    stop=(ci == CAPP - 1 and k == 3),
                                    skip_group_check=True)
                            for k in range(4):
                                nc.tensor.matmul(
                                    aggI_ps[:, k, :],
                                    hid2p[:, half, k * 128:(k + 1) * 128],
                                    sel_sb[:, c, :],
                                    start=(ci == 0 and k == 0),
                                    stop=(ci == CAPP - 1 and k == 3),
                                    skip_group_check=True)

                    if TRUNC >= 5:
                        nc.scalar.copy(aggHT[:, :, bi, :], aggH_ps[:])
                        nc.scalar.copy(aggIT[:, :, bi, :], aggI_ps[:])
                if DEBUG_DUMP and sb == 0:
                    nc.sync.dma_start(
                        dbg["d_aggHT"],
                        aggHT[:].rearrange("p a b c -> p (a b c)"))
                    nc.sync.dma_start(
                        dbg["d_aggIT"],
                        aggIT[:].rearrange("p a b c -> p (a b c)"))
                    nc.sync.dma_start(
                        dbg["d_gTb"],
                        gTb[:].rearrange("p a b c -> p (a b c)"))

                # ---- node MLP hidden (feature-major over 512 rows)
                h3 = wkb.tile([128, 4, D], V8, tag="h3")
                for g in range(4 if TRUNC >= 6 else 0):
                    gs = slice(g * 128, (g + 1) * 128)
                    ps3 = psA.tile([128, D], F32, tag="mm")
                    for k in range(4):
                        nc.tensor.matmul(ps3[:], w0a_sb[:, k, gs],
                                         gTb[:, :, k, :],
                                         start=(k == 0), stop=False)
                    if FP8_U:
                        for j in range(2):
                            jj = slice(2 * j, 2 * j + 2)
                            nc.tensor.matmul(ps3[:], u1_sb[:, jj, gs],
                                             aggHT[:, jj, :, :],
                                             perf_mode=DR,
                                             start=False, stop=False)
                        for j in range(2):
                            jj = slice(2 * j, 2 * j + 2)
                            nc.tensor.matmul(ps3[:], u2_sb[:, jj, gs],
                                             aggIT[:, jj, :, :],
                                             perf_mode=DR,
                                             start=False, stop=False)
                    else:
                        for k in range(4):
                            nc.tensor.matmul(ps3[:], u1_sb[:, k, gs],
                                             aggHT[:, k, :, :],
                                             start=False, stop=False)
                        for k in range(4):
                            nc.tensor.matmul(ps3[:], u2_sb[:, k, gs],
                                             aggIT[:, k, :, :],
                                             start=False, stop=False)
                    nc.tensor.matmul(ps3[:], v3b3_sb[:, gs], cnt_sb[:],
                                     start=False, stop=True)
                    nc.scalar.activation(h3[:, g, :], ps3[:], AF.Silu)

                # ---- pre4 = grid@ow0 + h3@V + b4 ; h4 = silu(pre4)
                h4 = wkb.tile([128, 4, D], BF16, tag="h4")
                for g in range(4 if TRUNC >= 7 else 0):
                    gs = slice(g * 128, (g + 1) * 128)
                    ps4 = psA.tile([128, D], F32, tag="mm")
                    for k in range(4):
                        nc.tensor.matmul(ps4[:], ow0_sb[:, k, gs],
                                         gTb[:, :, k, :],
                                         start=(k == 0), stop=False)
                    if FP8_V:
                        for j in range(2):
                            jj = slice(2 * j, 2 * j + 2)
                            nc.tensor.matmul(ps4[:], v_sb[:, jj, gs],
                                             h3[:, jj, :], perf_mode=DR,
                                             start=False,
                                             stop=(j == 1))
                    else:
                        for k in range(4):
                            nc.tensor.matmul(ps4[:], v_sb[:, k, gs],
                                             h3[:, k, :],
                                             start=False, stop=(k == 3))
                    if USE_ACTBIAS:
                        nc.scalar.activation(h4[:, g, :], ps4[:], AF.Silu,
                                             bias=b4c_sb[:, g:g + 1])
                    else:
                        t4 = wk.tile([128, D], BF16, tag="t4")
                        nc.vector.tensor_scalar(t4[:], ps4[:],
                                                b4c_sb[:, g:g + 1], None,
                                                op0=ALU.add)
                        nc.scalar.activation(h4[:, g, :], t4[:], AF.Silu)
                if DEBUG_DUMP and sb == 0:
                    nc.sync.dma_start(
                        dbg["d_h3"], h3[:].rearrange("p a b -> p (a b)"))
                    nc.sync.dma_start(
                        dbg["d_h4"], h4[:].rearrange("p a b -> p (a b)"))

                # ---- out = h4 @ ow1 + ob1  (row-major out)
                for sc in range(4 if TRUNC >= 7 else 0):
                    rs = slice(sc * 128, (sc + 1) * 128)
                    pso = psA.tile([128, OUTD], F32, tag="mm")
                    for g in range(4):
                        nc.tensor.matmul(pso[:], h4[:, g, rs],
                                         ow1_sb[:, g, :],
                                         start=(g == 0), stop=(g == 3))
                    ot = io3.tile([128, OUTD], F32, tag="ot")
                    nc.vector.tensor_add(ot[:], pso[:], ob1_sb[:])
                    nc.sync.dma_start(
                        outt[sb * 512 + sc * 128:sb * 512 + (sc + 1) * 128, :],
                        ot[:])

    from concourse.library_overlay import lower_extended_insts
    lower_extended_insts(nc)
    if SPLIT_WAITS:
        _split_multi_waits(nc)
    return nc


def _split_multi_waits(nc):
    """This walrus build allows at most ONE sync wait per instruction.
    Move surplus waits onto EventSemaphore carrier instructions inserted
    immediately before, on the same engine."""
    for f in nc.m.functions:
        for bb in f.blocks:
            insts = list(bb.instructions)
            if not any(i.sync_info is not None and len(i.sync_info.on_wait) > 1
                       for i in insts):
                continue
            new = []
            for ins in insts:
                si = ins.sync_info
                if si is not None and len(si.on_wait) > 1:
                    waits = list(si.on_wait)
                    for w in waits[:-1]:
                        c = mybir.InstEventSemaphore(
                            name=f"I-w{nc.next_id()}", engine=ins.engine,
                            ins=[], outs=[],
                            sync_info=mybir.SyncInfo(on_wait=[w], on_update=[]))
                        new.append(c)
                    del si.on_wait[:]
                    si.on_wait.append(waits[-1])
                new.append(ins)
            bb.instructions = new


# ------------------------------------------------------------ host pipeline
def _pack_rows(deg, nbins, cap_edges):
    """Assign nbins*128 rows (with edge degrees `deg`) to nbins bins of
    exactly 128 rows such that each bin's total degree <= cap_edges.
    Greedy: rows by descending degree into the min-load bin with row room.
    Returns the bin assignment or None if the cap is infeasible."""
    import heapq
    order = np.argsort(-deg, kind="stable")
    assign = np.empty(nbins * 128, np.int64)
    heap = [(0, 0, b) for b in range(nbins)]
    heapq.heapify(heap)
    for r in order:
        while True:
            e, rr, b = heapq.heappop(heap)
            if rr < 128:
                break
        if e + deg[r] > cap_edges:
            return None
        assign[r] = b
        heapq.heappush(heap, (e + int(deg[r]), rr + 1, b))
    return assign


def _prep(inputs):
    """Host-side weight folding + edge packing. Returns (in_maps, CAPP)."""
    mesh_f = np.asarray(inputs["mesh_node_features"])[0]   # [N_MESH, D]
    grid_f = np.asarray(inputs["grid_node_features"])[0]   # [N_GRID, D]
    attrs = np.asarray(inputs["edge_attrs"])               # [E, 4]
    esrc = np.asarray(inputs["edge_src"]).astype(np.int64)
    edst = np.asarray(inputs["edge_dst"]).astype(np.int64)

    # ---- fold weights (fp32 on host)
    W = {k: np.asarray(inputs[k], np.float32) for k in (
        "emb_w0", "emb_b0", "emb_w1", "emb_b1", "edge_w0", "edge_b0",
        "edge_w1", "edge_b1", "node_w0", "node_b0", "node_w1", "node_b1",
        "out_w0", "out_b0", "out_w1", "out_b1")}
    Ws, Wd, We = W["edge_w0"][:D], W["edge_w0"][D:2 * D], W["edge_w0"][2 * D:]
    W0a, W0b = W["node_w0"][:D], W["node_w0"][D:]
    W_he = W["emb_w1"] @ We
    b2 = W["emb_b1"] @ We + W["edge_b0"]
    U1 = W["emb_w1"] @ W0b
    U2 = W["edge_w1"] @ W0b
    v3 = (W["emb_b1"] + W["edge_b1"]) @ W0b
    V = W["node_w1"] @ W["out_w0"]
    b4 = W["node_b1"] @ W["out_w0"] + W["out_b0"]
    emb_w0b = np.concatenate([W["emb_w0"], W["emb_b0"][None]], 0)  # [5, D]
    v3b3 = np.stack([v3, W["node_b0"]], 0)                          # [2, D]

    # ---- global row -> (core, block) bin packing. Rows are freely
    # permutable (host unpermutes the output), so balance shard load and
    # per-block edge caps in one 512-bin pack.
    NBINS = NCORES * NB
    NRT = NBINS * 128                     # 65536 row slots
    deg = np.bincount(edst, minlength=NRT)
    CAP = max(2, int(math.ceil(len(edst) / (NBINS * 128.0))))
    while True:
        assign = _pack_rows(deg, NBINS, CAP * 128)
        if assign is not None:
            break
        CAP += 1
    CAPP = CAP + (CAP % 2) if FP8_SCATTER else CAP
    NCH = NB * CAPP
    ECP = NCH * 128
    # global packed position of each row; bins are filled to exactly 128
    order_by_bin = np.argsort(assign, kind="stable")
    pos_global = np.empty(NRT, np.int64)
    pos_global[order_by_bin] = np.arange(NRT)
    nd_global = pos_global[edst]
    core_of = nd_global // NGS

    w8 = f8 if FP8_WHE else bf
    u8 = f8 if FP8_U else bf
    v8 = f8 if FP8_V else bf
    meshTc_full = _chunkT(np.concatenate(
        [mesh_f, np.zeros((NM - N_MESH, D), np.float32)]).astype(bf))
    shared = {
        "w_ws": Ws.astype(bf), "w_wd": Wd.astype(bf),
        "w_whe": W_he.astype(w8), "w_emb0": emb_w0b.astype(bf),
        "w_u1": U1.astype(u8), "w_u2": U2.astype(u8),
        "w_w0a": W0a.astype(bf), "w_ow0": W["out_w0"].astype(bf),
        "w_v": V.astype(v8), "w_ow1": W["out_w1"].astype(bf),
        "v3b3": v3b3.astype(bf),
        "b2bc": np.ascontiguousarray(
            np.broadcast_to(b2[None], (128, D))).astype(bf),
        "b4col": np.ascontiguousarray(
            b4.reshape(4, 128).T).astype(np.float32),
        "ob1bc": np.ascontiguousarray(
            np.broadcast_to(W["out_b1"][None], (128, OUTD))).astype(
                np.float32),
    }

    grid_ext = np.zeros((NRT, D), np.float32)
    grid_ext[:N_GRID] = grid_f

    in_maps = []
    for core in range(NCORES):
        m = core_of == core
        cs, ca = esrc[m], attrs[m]
        nd = nd_global[m] - core * NGS       # local packed destination
        eo = np.argsort(nd, kind="stable")   # group edges by block
        cs, nd, ca = cs[eo], nd[eo], ca[eo]
        cb = nd // 128
        src_p = np.zeros(ECP, np.int16)
        att_p = np.zeros((ECP, 4), np.float32)
        nbc = np.bincount(cb, minlength=NB)
        assert nbc.max() <= CAPP * 128
        starts = np.arange(NB) * CAPP * 128
        pos_in_blk = np.arange(len(cs)) - np.repeat(
            np.cumsum(nbc) - nbc, nbc)
        slot = starts[cb] + pos_in_blk
        src_p[slot] = cs
        att_p[slot] = ca
        attrsT5 = np.concatenate(
            [att_p.T, np.ones((1, ECP), np.float32)], 0).astype(bf)
        e_lo = slot % 128
        ch = slot // 128
        d_lo = nd - cb * 128
        sel = np.zeros((128, NCH, 128), f8)
        sel[e_lo, ch, d_lo] = 1.0
        selT = np.zeros((128, NCH, 128), f8)
        selT[d_lo, ch, e_lo] = 1.0
        grid_perm = grid_ext[order_by_bin[core * NGS:(core + 1) * NGS]]
        cnt = np.zeros(NGS, np.float32)
        np.add.at(cnt, nd, 1.0)
        cntones = np.stack([cnt, np.ones(NGS, np.float32)], 0).astype(bf)
        mtc = (meshTc_full[:, core * NMC_L * 512:(core + 1) * NMC_L * 512]
               if SHARD_P1 else meshTc_full)
        in_maps.append(dict(shared,
                            meshTc=np.ascontiguousarray(mtc),
                            gridTc=_chunkT(grid_perm.astype(bf)),
                            attrsT5=np.ascontiguousarray(attrsT5),
                            srcidx=_wrap_idx(src_p),
                            sel=sel, selT=selT,
                            cntones=cntones,
                            _pos_global=pos_global))
    return in_maps, CAPP


_CACHE = {}


class _Runner:
    """Persistent jitted SPMD executor (avoids re-jitting per call)."""

    def __init__(self, nc):
        import jax
        from jax.experimental.shard_map import shard_map
        from jax.sharding import Mesh, PartitionSpec
        from concourse import bass2jax

        bass2jax.install_neuronx_cc_hook()
        self.nc = nc
        part_name = (nc.partition_id_tensor.name
                     if nc.partition_id_tensor else None)
        in_names, out_names, out_avals, zero_outs = [], [], [], []
        for alloc in nc.m.functions[0].allocations:
            if not isinstance(alloc, mybir.MemoryLocationSet):
                continue
            name = alloc.memorylocations[0].name
            if alloc.kind == "ExternalInput":
                if name != part_name:
                    in_names.append(name)
            elif alloc.kind == "ExternalOutput":
                shape = tuple(alloc.tensor_shape)
                dtype = mybir.dt.np(alloc.dtype)
                out_names.append(name)
                out_avals.append(jax.core.ShapedArray(shape, dtype))
                zero_outs.append(np.zeros(shape, dtype))
        self.in_names = list(in_names)
        self.out_names = out_names
        self.out_shapes = [tuple(a.shape) for a in out_avals]
        all_names = in_names + out_names
        if part_name is not None:
            all_names = all_names + [part_name]

        def _body(*args):
            operands = list(args)
            if part_name is not None:
                operands.append(bass2jax.partition_id_tensor())
            outs = bass2jax._bass_exec_p.bind(
                *operands,
                out_avals=tuple(out_avals),
                in_names=tuple(all_names),
                out_names=tuple(out_names),
                lowering_input_output_aliases=(),
                sim_require_finite=True,
                sim_require_nnan=True,
                nc=nc,
            )
            return tuple(outs)

        devices = jax.devices()[:NCORES]
        mesh = Mesh(np.asarray(devices), ("core",))
        nin = len(self.in_names) + len(out_names)
        self.fn = jax.jit(shard_map(
            _body, mesh=mesh,
            in_specs=(PartitionSpec("core"),) * nin,
            out_specs=(PartitionSpec("core"),) * len(out_names),
            check_rep=False))
        self.zero_outs = zero_outs
        self.sharding = jax.sharding.NamedSharding(mesh, PartitionSpec("core"))
        self.mesh = mesh
        self._avals = out_avals
        self._jax = jax

    def put(self, in_maps):
        arrs = []
        for name in self.in_names:
            arrs.append(np.concatenate([m[name] for m in in_maps], axis=0))
        for z in self.zero_outs:
            arrs.append(np.concatenate([z] * NCORES, axis=0))
        return [self._jax.device_put(a, self.sharding) for a in arrs]

    def run(self, arrs):
        return self.fn(*arrs)

    def get(self, outs):
        res = [np.asarray(o) for o in outs]
        per_core = []
        for c in range(NCORES):
            d = {}
            for i, name in enumerate(self.out_names):
                n0 = self.out_shapes[i][0]
                d[name] = res[i][c * n0:(c + 1) * n0]
            per_core.append(d)
        return per_core


def _get_runner(CAPP) -> _Runner:
    if CAPP not in _CACHE:
        _CACHE[CAPP] = _Runner(build_bass(CAPP))
    return _CACHE[CAPP]


def kernel(**inputs) -> np.ndarray:
    in_maps, CAPP = _prep(inputs)
    r = _get_runner(CAPP)
    outs = r.run(r.put(in_maps))
    per_core = r.get(outs)
    # rows were bin-packed into (core, block) bins on device; unpermute
    big = np.concatenate([per_core[c]["outt"] for c in range(NCORES)], axis=0)
    out = big[in_maps[0]["_pos_global"][:N_GRID]]
    return out[None].astype(np.float32)
